# revision 15
# baseline (speedup 1.0000x reference)
# Trainium2 Bass kernel for: ConvTranspose2d(64->128, k=4, stride=1) -> spatial
# mean -> +biases -> 10*logsumexp over channels.
#
# Math: with full (K-1) output padding, the mean over the ENTIRE conv-transpose
# output spatial extent sees every input pixel through all K*K taps, so
#   pooled[n,co] = (sum_hw x[n,ci,hw]) @ (sum_kk w[ci,co,kk]) / (Ho*Wo) + cb + eb
# exactly. The conv collapses to a spatial sum + a (Cin x Cout) matmul.
#
# Sharding: data-parallel over batch N=32 across 8 cores (4 batches/core),
# params replicated. Each core streams its 4 MiB x-slice (8 chunked HWDGE DMAs
# on the SP ring), reduces spatially on DVE, runs two tiny accumulating PE
# matmuls (bias + data), then exp-accumulate + log on ACT.
#
# Layout trick: the spatial sums land as s2[p, r] with p = (n%2)*64 + ci,
# r = n//2 (two 128-row blocks of x). Instead of shuffling partitions, the
# weight-sum matrix is duplicated onto both partition halves (tiny SBUF->SBUF
# DMA) and s2 is expanded into a zero-masked (128, 4) lhsT so a single
# 128-deep matmul contracts each batch against its own partition half.
#
# Trace-driven choices (see test.py profiling):
# - weight/bias DMAs ride the ACT HWDGE ring, parallel to x on the SP ring;
#   a SWDGE broadcast DMA would split by the dup dim onto 2 SDMA engines and
#   starve the x stream.
# - all spatial reduces stay on DVE: mixing in ACT Copy+accum ops evicts the
#   Exp/Ln activation tables and puts two 1.3us ACT_TABLE_LOADs on the tail.
# - one pre-placed LoadActFuncSet covering BOTH Exp and Ln runs at kernel
#   start, so no ACT_TABLE_LOAD lands on the critical tail.

import os

import numpy as np

import concourse.bacc as bacc
import concourse.bass as bass
import concourse.mybir as mybir
import concourse.tile as tile
from concourse.bass_utils import run_bass_kernel_spmd
from concourse.hw_specs import get_activation_tables

N, CIN, COUT, K, H, W = 32, 64, 128, 4, 64, 64
NCORES = 8
NLOC = N // NCORES          # 4 batches per core
HW = H * W                  # 4096
ROWS = NLOC * CIN           # 256 rows (n,ci) per core
RBLK = ROWS // 128          # 2 row blocks of 128 partitions
CHUNK = 1024                # column chunk width (512 KiB per DMA)
NCHUNK = HW // CHUNK        # 4
SCALE = 1.0 / float((H + K - 1) * (W + K - 1))   # 1/4489

F32 = mybir.dt.float32

_CACHE: dict = {}


def _build_module() -> bacc.Bacc:
    nc = bacc.Bacc("TRN2", target_bir_lowering=False, enable_partition_id=False)

    x_d = nc.dram_tensor("xc", [ROWS, HW], F32, kind="ExternalInput").ap()
    w_d = nc.dram_tensor("w", [CIN, COUT * K * K], F32, kind="ExternalInput").ap()
    bs_d = nc.dram_tensor("bs", [2, COUT], F32, kind="ExternalInput").ap()
    y_d = nc.dram_tensor("y", [NLOC, 1], F32, kind="ExternalOutput").ap()

    with tile.TileContext(nc) as tc:
        with (
            tc.tile_pool(name="xpool", bufs=RBLK * NCHUNK) as xpool,
            tc.tile_pool(name="small", bufs=1) as small,
            tc.tile_pool(name="psum", bufs=1, space="PSUM") as psum_pool,
        ):
            # preload the one ACT table set that covers BOTH Exp and Ln
            # ("natural_log_exp_and_others"), so insert_act_table_loads has
            # nothing to add and no 1.3us ACT_TABLE_LOAD lands on the tail
            # between exp and ln (the pass's per-use set choice would pick
            # two different sets and reload mid-chain otherwise).
            act_tables = get_activation_tables(nc.m.arch)
            set_id = next(
                i
                for i, (_, funcs) in enumerate(act_tables.items())
                if mybir.ActivationFunctionType.Exp in funcs
                and mybir.ActivationFunctionType.Ln in funcs
            )
            nc.scalar.add_instruction(
                mybir.InstLoadActFuncSet(
                    name=nc.get_next_instruction_name(), act_func_set_id=set_id
                )
            )

            # ---- params ----
            wk = small.tile([CIN, COUT * K * K], F32)
            nc.scalar.dma_start(out=wk, in_=w_d)
            wdup = small.tile([128, COUT], F32)
            nc.vector.reduce_sum(
                out=wdup[0:CIN, :],
                in_=wk.rearrange("p (c k) -> p c k", k=K * K),
                axis=mybir.AxisListType.X,
            )
            # fold the 1/(Ho*Wo) mean scale into the weight sums
            nc.vector.tensor_scalar_mul(
                out=wdup[0:CIN, :], in0=wdup[0:CIN, :], scalar1=SCALE
            )
            # duplicate onto the other partition half (32 KB on-chip copy)
            nc.scalar.dma_start(out=wdup[CIN:128, :], in_=wdup[0:CIN, :])

            biasrows = small.tile([2, COUT], F32)
            nc.scalar.dma_start(out=biasrows, in_=bs_d)
            onesb = small.tile([2, NLOC], F32)
            nc.vector.memset(onesb, 1.0)

            # ---- spatial sums of x (all on DVE) ----
            # the final chunks taper down so the last reduce (which gates the
            # whole tail) is short
            chunk_plan = {
                r: ([CHUNK] * NCHUNK if r < RBLK - 1 else [CHUNK] * (NCHUNK - 1) + [CHUNK // 2, CHUNK // 2])
                for r in range(RBLK)
            }
            ncols = sum(len(v) for v in chunk_plan.values())
            parts = small.tile([128, RBLK, NCHUNK + 1], F32)
            nc.vector.memset(parts, 0.0)
            for r in range(RBLK):
                col0 = 0
                for ci, w in enumerate(chunk_plan[r]):
                    xt = xpool.tile([128, w], F32, tag=f"xt{w}")
                    nc.sync.dma_start(
                        out=xt,
                        in_=x_d[r * 128 : (r + 1) * 128, col0 : col0 + w],
                    )
                    nc.vector.reduce_sum(
                        out=parts[:, r, ci : ci + 1],
                        in_=xt,
                        axis=mybir.AxisListType.X,
                    )
                    col0 += w
            s2 = small.tile([128, RBLK], F32)
            nc.vector.reduce_sum(out=s2, in_=parts, axis=mybir.AxisListType.X)

            # ---- masked lhsT (128, 4): col n nonzero only on its own half ----
            # s2m[(n%2)*64 + ci, n] = S[n, ci], zeros elsewhere
            s2m = small.tile([128, NLOC], F32)
            nc.vector.memset(s2m, 0.0)
            s2m_v = s2m.rearrange("p (r t) -> p r t", t=2)  # [p, r, halfsel]
            s2_v = s2.rearrange("p (r t) -> p r t", t=1)  # [p, r, 1]
            nc.vector.tensor_copy(s2m_v[0:64, :, 0:1], s2_v[0:64, :, :])
            nc.vector.tensor_copy(s2m_v[64:128, :, 1:2], s2_v[64:128, :, :])

            # ---- pooled^T (4, 128) in PSUM: bias matmul + data matmul ----
            pooled = psum_pool.tile([NLOC, COUT], F32, space="PSUM")
            nc.tensor.matmul(
                out=pooled, lhsT=onesb, rhs=biasrows, start=True, stop=False
            )
            nc.tensor.matmul(out=pooled, lhsT=s2m, rhs=wdup, start=False, stop=True)

            # ---- 10 * log(sum_co exp(pooled)) ----
            expt = small.tile([NLOC, COUT], F32)
            sume = small.tile([NLOC, 1], F32)
            nc.scalar.activation(
                out=expt,
                in_=pooled,
                func=mybir.ActivationFunctionType.Exp,
                accum_out=sume,
            )
            logv = small.tile([NLOC, 1], F32)
            nc.scalar.activation(
                out=logv, in_=sume, func=mybir.ActivationFunctionType.Ln
            )
            outv = small.tile([NLOC, 1], F32)
            nc.scalar.mul(out=outv, in_=logv, mul=10.0)
            # issue the output DMA from ACT itself: no cross-engine hop
            nc.scalar.dma_start(out=y_d, in_=outv)

    nc.compile()
    return nc


def kernel(x, weight, conv_bias, extra_bias):
    x = np.ascontiguousarray(np.asarray(x, dtype=np.float32))
    weight = np.ascontiguousarray(np.asarray(weight, dtype=np.float32))
    conv_bias = np.ascontiguousarray(np.asarray(conv_bias, dtype=np.float32))
    extra_bias = np.ascontiguousarray(np.asarray(extra_bias, dtype=np.float32))
    assert x.shape == (N, CIN, H, W), x.shape
    assert weight.shape == (CIN, COUT, K, K), weight.shape

    if "nc" not in _CACHE:
        _CACHE["nc"] = _build_module()
    nc = _CACHE["nc"]

    w2 = weight.reshape(CIN, COUT * K * K)
    bs2 = np.ascontiguousarray(
        np.stack([conv_bias, extra_bias], axis=0)
    )  # (2, COUT)
    in_maps = []
    for c in range(NCORES):
        xc = x[c * NLOC : (c + 1) * NLOC].reshape(ROWS, HW)
        in_maps.append({"xc": xc, "w": w2, "bs": bs2})

    trace = os.environ.get("BASS_KERNEL_TRACE") == "1"
    res = run_bass_kernel_spmd(
        nc, in_maps, core_ids=list(range(NCORES)), trace=trace
    )
    _CACHE["last_result"] = res
    return np.concatenate([r["y"] for r in res.results], axis=0)
